# revision 1
# baseline (speedup 1.0000x reference)
"""Trainium2 Bass kernel for a debiased GRU cell.

Computation (per batch row):
    r   = sigmoid(W_r @ [x; h] + b_r)
    u   = sigmoid(W_u @ [x; h] + b_u)
    hh  = tanh(W_h @ [x_int; r*h] + b_h)
    s   = score * u
    out = (1 - s) * hh + s * h

Strategy: data-parallel over 8 cores (8192 rows each). On-chip layout is
feature-major ([H, batch]) so that
  - activations never need an on-chip transpose (host supplies x.T / h.T),
  - gate biases fuse into the ACT engine's per-partition bias operand,
  - matmuls run with full K=128 / M=128 / N=512 tiles (PE at peak rate).
The only broadcast needed (attention score along partitions) runs on the
otherwise-idle GPSIMD engine. Output is produced as out.T and un-transposed
on the host.
"""

import numpy as np

import concourse.bacc as bacc
import concourse.bass as bass
import concourse.mybir as mybir
import concourse.tile as tile
from concourse.bass_utils import run_bass_kernel_spmd

B = 65536
I = 256
H = 256
NCORES = 8
BC = B // NCORES  # rows per core
NB = 512          # batch columns per block (max fp32 matmul free dim)
NBLK = BC // NB   # 16
FP32 = mybir.dt.float32
AF = mybir.ActivationFunctionType

_NC_CACHE = {}


def _build_nc():
    nc = bacc.Bacc(
        "TRN2",
        target_bir_lowering=False,
        debug=False,
        enable_asserts=False,
    )

    xT = nc.dram_tensor("xT", [2 * I, BC], FP32, kind="ExternalInput")
    hT = nc.dram_tensor("hT", [H, BC], FP32, kind="ExternalInput")
    sc = nc.dram_tensor("sc", [NBLK, 1, NB], FP32, kind="ExternalInput")
    wg = nc.dram_tensor("wg", [128, 24 * 128], FP32, kind="ExternalInput")
    wh = nc.dram_tensor("wh", [128, 8 * 128], FP32, kind="ExternalInput")
    bg = nc.dram_tensor("bg", [128, 4], FP32, kind="ExternalInput")
    bh = nc.dram_tensor("bh", [128, 2], FP32, kind="ExternalInput")
    outT = nc.dram_tensor("outT", [H, BC], FP32, kind="ExternalOutput")

    # [blk, partition, k-chunk, col]
    xTr = xT.rearrange("(k p) (b n) -> b p k n", p=128, n=NB)
    hTr = hT.rearrange("(k p) (b n) -> b p k n", p=128, n=NB)
    outTr = outT.rearrange("(m p) (b n) -> b p m n", p=128, n=NB)

    with tile.TileContext(nc) as tc:
        with (
            tc.tile_pool(name="const", bufs=1) as cpool,
            tc.tile_pool(name="xin", bufs=3) as xpool,
            tc.tile_pool(name="hin", bufs=3) as hpool,
            tc.tile_pool(name="sin", bufs=3) as spool,
            tc.tile_pool(name="gates", bufs=2) as gpool,
            tc.tile_pool(name="work", bufs=2) as wpool,
            tc.tile_pool(name="outp", bufs=2) as opool,
            tc.tile_pool(name="psg", bufs=3, space=bass.MemorySpace.PSUM) as pgpool,
            tc.tile_pool(name="psh", bufs=1, space=bass.MemorySpace.PSUM) as phpool,
        ):
            wg_sb = cpool.tile([128, 24 * 128], FP32)
            nc.sync.dma_start(wg_sb[:], wg[:])
            wh_sb = cpool.tile([128, 8 * 128], FP32)
            nc.sync.dma_start(wh_sb[:], wh[:])
            bg_sb = cpool.tile([128, 4], FP32)
            nc.sync.dma_start(bg_sb[:], bg[:])
            bh_sb = cpool.tile([128, 2], FP32)
            nc.sync.dma_start(bh_sb[:], bh[:])

            def emit_gates(b):
                """Load block b, run gate matmuls + sigmoids + r*h."""
                xt = xpool.tile([128, 4, NB], FP32, tag="xt")
                nc.sync.dma_start(xt[:], xTr[b])
                ht = hpool.tile([128, 2, NB], FP32, tag="ht")
                nc.sync.dma_start(ht[:], hTr[b])
                srow = spool.tile([1, NB], FP32, tag="srow")
                nc.sync.dma_start(srow[:], sc[b])
                sbc = spool.tile([128, 2, NB], FP32, tag="sbc")
                nc.gpsimd.partition_broadcast(sbc[:, 0, :], srow[:])
                nc.gpsimd.partition_broadcast(sbc[:, 1, :], srow[:])

                pg_r = pgpool.tile([128, 2, NB], FP32, tag="pg")
                pg_u = pgpool.tile([128, 2, NB], FP32, tag="pg")
                for gi in range(4):  # r0, r1, u0, u1
                    dst = (pg_r if gi < 2 else pg_u)[:, gi % 2, :]
                    for k in range(6):
                        act = xt[:, k, :] if k < 4 else ht[:, k - 4, :]
                        c = gi * 6 + k
                        nc.tensor.matmul(
                            dst,
                            wg_sb[:, c * 128:(c + 1) * 128],
                            act,
                            start=(k == 0),
                            stop=(k == 5),
                        )
                r = gpool.tile([128, 2, NB], FP32, tag="r")
                u = gpool.tile([128, 2, NB], FP32, tag="u")
                for m in range(2):
                    nc.scalar.activation(
                        r[:, m, :], pg_r[:, m, :], AF.Sigmoid, bias=bg_sb[:, m:m + 1]
                    )
                    nc.scalar.activation(
                        u[:, m, :], pg_u[:, m, :], AF.Sigmoid, bias=bg_sb[:, 2 + m:3 + m]
                    )
                rh = wpool.tile([128, 2, NB], FP32, tag="rh")
                nc.vector.tensor_mul(rh[:], r[:], ht[:])
                return dict(xt=xt, ht=ht, sbc=sbc, u=u, rh=rh)

            def emit_h(b, st):
                """h_hat matmul + tanh + final combine + store for block b."""
                ph = phpool.tile([128, 2, NB], FP32, tag="ph")
                for m in range(2):
                    for k in range(4):
                        act = st["xt"][:, k, :] if k < 2 else st["rh"][:, k - 2, :]
                        c = m * 4 + k
                        nc.tensor.matmul(
                            ph[:, m, :],
                            wh_sb[:, c * 128:(c + 1) * 128],
                            act,
                            start=(k == 0),
                            stop=(k == 3),
                        )
                hhat = wpool.tile([128, 2, NB], FP32, tag="hhat")
                for m in range(2):
                    nc.scalar.activation(
                        hhat[:, m, :], ph[:, m, :], AF.Tanh, bias=bh_sb[:, m:m + 1]
                    )
                e1 = wpool.tile([128, 2, NB], FP32, tag="e1")
                nc.vector.tensor_sub(e1[:], st["ht"][:], hhat[:])
                e2 = wpool.tile([128, 2, NB], FP32, tag="e2")
                nc.vector.tensor_mul(e2[:], st["u"][:], st["sbc"][:])
                t = wpool.tile([128, 2, NB], FP32, tag="t")
                nc.vector.tensor_mul(t[:], e1[:], e2[:])
                o = opool.tile([128, 2, NB], FP32, tag="o")
                nc.vector.tensor_add(o[:], t[:], hhat[:])
                nc.sync.dma_start(outTr[b], o[:])

            # Software-pipelined emission: block b's h-chain is emitted after
            # block b+1's gate matmuls so the PE never waits on the r*h
            # elementwise product.
            prev = None
            for b in range(NBLK):
                st = emit_gates(b)
                if prev is not None:
                    emit_h(b - 1, prev)
                prev = st
            emit_h(NBLK - 1, prev)

    nc.compile()
    return nc


def _get_nc():
    if "nc" not in _NC_CACHE:
        _NC_CACHE["nc"] = _build_nc()
    return _NC_CACHE["nc"]


def _pack_weights(W_r, W_u, W_h, b_r, b_u, b_h):
    wg = np.empty((128, 24 * 128), np.float32)
    for gi in range(4):
        W = W_r if gi < 2 else W_u
        m = gi % 2
        for k in range(6):
            c = gi * 6 + k
            wg[:, c * 128:(c + 1) * 128] = W[m * 128:(m + 1) * 128,
                                             k * 128:(k + 1) * 128].T
    wh = np.empty((128, 8 * 128), np.float32)
    for m in range(2):
        for k in range(4):
            c = m * 4 + k
            wh[:, c * 128:(c + 1) * 128] = W_h[m * 128:(m + 1) * 128,
                                               k * 128:(k + 1) * 128].T
    bg = np.stack([b_r[:128], b_r[128:], b_u[:128], b_u[128:]], axis=1)
    bh = np.stack([b_h[:128], b_h[128:]], axis=1)
    return (np.ascontiguousarray(wg), np.ascontiguousarray(wh),
            np.ascontiguousarray(bg), np.ascontiguousarray(bh))


def _make_in_maps(inputs, h_prev, attention_score, W_r, b_r, W_u, b_u, W_h, b_h):
    inputs = np.asarray(inputs, np.float32)
    h_prev = np.asarray(h_prev, np.float32)
    attention_score = np.asarray(attention_score, np.float32)
    wg, wh, bg, bh = _pack_weights(
        np.asarray(W_r, np.float32), np.asarray(W_u, np.float32),
        np.asarray(W_h, np.float32), np.asarray(b_r, np.float32),
        np.asarray(b_u, np.float32), np.asarray(b_h, np.float32),
    )
    in_maps = []
    for c in range(NCORES):
        sl = slice(c * BC, (c + 1) * BC)
        in_maps.append({
            "xT": np.ascontiguousarray(inputs[sl].T),
            "hT": np.ascontiguousarray(h_prev[sl].T),
            "sc": np.ascontiguousarray(attention_score[sl].reshape(NBLK, 1, NB)),
            "wg": wg, "wh": wh, "bg": bg, "bh": bh,
        })
    return in_maps


def _run(in_maps, trace=False, **kwargs):
    return run_bass_kernel_spmd(
        _get_nc(), in_maps, core_ids=list(range(NCORES)), trace=trace, **kwargs
    )


def _gather(results):
    out = np.empty((B, H), np.float32)
    for c in range(NCORES):
        out[c * BC:(c + 1) * BC] = results[c]["outT"].T
    return out


def kernel(**inputs):
    res = _run(_make_in_maps(**inputs), trace=False)
    return _gather(res.results)


# revision 5
# speedup vs baseline: 1.9441x; 1.9441x over previous
"""Trainium2 Bass kernel for a debiased GRU cell.

Computation (per batch row):
    r   = sigmoid(W_r @ [x; h] + b_r)
    u   = sigmoid(W_u @ [x; h] + b_u)
    hh  = tanh(W_h @ [x_int; r*h] + b_h)
    s   = score * u
    out = (1 - s) * hh + s * h

Strategy: data-parallel over 8 cores (8192 rows each). On-chip layout is
feature-major ([H, batch]) so that
  - activations never need an on-chip transpose (host supplies x.T / h.T),
  - gate biases fuse into the ACT engine's per-partition bias operand,
  - matmuls run with full K=128 / M=128 / N=512 tiles (PE at peak rate).
The only broadcast needed (attention score along partitions) runs on the
otherwise-idle GPSIMD engine. Output is produced as out.T and un-transposed
on the host.
"""

import numpy as np

import concourse.bacc as bacc
import concourse.bass as bass
import concourse.mybir as mybir
import concourse.tile as tile
from concourse.bass_utils import run_bass_kernel_spmd

B = 65536
I = 256
H = 256
NCORES = 8
BC = B // NCORES  # rows per core
NB = 512          # batch columns per block (max fp32 matmul free dim)
NBLK = BC // NB   # 16
FP32 = mybir.dt.float32
AF = mybir.ActivationFunctionType

_NC_CACHE = {}


def _build_nc(reps=1):
    nc = bacc.Bacc(
        "TRN2",
        target_bir_lowering=False,
        debug=False,
        enable_asserts=False,
    )

    xT = nc.dram_tensor("xT", [2 * I, BC], FP32, kind="ExternalInput")
    hT = nc.dram_tensor("hT", [H, BC], FP32, kind="ExternalInput")
    sc = nc.dram_tensor("sc", [NBLK, 1, NB], FP32, kind="ExternalInput")
    wg = nc.dram_tensor("wg", [128, 24 * 128], FP32, kind="ExternalInput")
    wh = nc.dram_tensor("wh", [128, 8 * 128], FP32, kind="ExternalInput")
    bg = nc.dram_tensor("bg", [128, 4], FP32, kind="ExternalInput")
    bh = nc.dram_tensor("bh", [128, 2], FP32, kind="ExternalInput")
    outT = nc.dram_tensor("outT", [H, BC], FP32, kind="ExternalOutput")

    # [blk, partition, k-chunk, col]
    xTr = xT.rearrange("(k p) (b n) -> b p k n", p=128, n=NB)
    hTr = hT.rearrange("(k p) (b n) -> b p k n", p=128, n=NB)
    outTr = outT.rearrange("(m p) (b n) -> b p m n", p=128, n=NB)

    with tile.TileContext(nc) as tc:
        with (
            tc.tile_pool(name="const", bufs=1) as cpool,
            tc.tile_pool(name="xin", bufs=3) as xpool,
            tc.tile_pool(name="hin", bufs=3) as hpool,
            tc.tile_pool(name="sin", bufs=3) as spool,
            tc.tile_pool(name="gates", bufs=2) as gpool,
            tc.tile_pool(name="work", bufs=2) as wpool,
            tc.tile_pool(name="outp", bufs=2) as opool,
            tc.tile_pool(name="psg", bufs=3, space=bass.MemorySpace.PSUM) as pgpool,
            tc.tile_pool(name="psh", bufs=1, space=bass.MemorySpace.PSUM) as phpool,
        ):
            wg_sb = cpool.tile([128, 24 * 128], FP32)
            nc.sync.dma_start(wg_sb[:], wg[:])
            wh_sb = cpool.tile([128, 8 * 128], FP32)
            nc.sync.dma_start(wh_sb[:], wh[:])
            bg_sb = cpool.tile([128, 4], FP32)
            nc.sync.dma_start(bg_sb[:], bg[:])
            bh_sb = cpool.tile([128, 2], FP32)
            nc.sync.dma_start(bh_sb[:], bh[:])

            def emit_gates(b):
                """Load block b, run gate matmuls + sigmoids + r*h."""
                xt = xpool.tile([128, 4, NB], FP32, tag="xt")
                nc.sync.dma_start(xt[:], xTr[b])
                ht = hpool.tile([128, 2, NB], FP32, tag="ht")
                nc.sync.dma_start(ht[:], hTr[b])
                srow = spool.tile([1, NB], FP32, tag="srow")
                nc.sync.dma_start(srow[:], sc[b])
                sbc = spool.tile([128, 2, NB], FP32, tag="sbc")
                nc.gpsimd.partition_broadcast(sbc[:, 0, :], srow[:])
                nc.gpsimd.partition_broadcast(sbc[:, 1, :], srow[:])

                pg_r = pgpool.tile([128, 2, NB], FP32, tag="pg")
                pg_u = pgpool.tile([128, 2, NB], FP32, tag="pg")
                for gi in range(4):  # r0, r1, u0, u1
                    dst = (pg_r if gi < 2 else pg_u)[:, gi % 2, :]
                    for k in range(6):
                        act = xt[:, k, :] if k < 4 else ht[:, k - 4, :]
                        c = gi * 6 + k
                        nc.tensor.matmul(
                            dst,
                            wg_sb[:, c * 128:(c + 1) * 128],
                            act,
                            start=(k == 0),
                            stop=(k == 5),
                        )
                r = gpool.tile([128, 2, NB], FP32, tag="r")
                u = gpool.tile([128, 2, NB], FP32, tag="u")
                for m in range(2):
                    nc.scalar.activation(
                        r[:, m, :], pg_r[:, m, :], AF.Sigmoid, bias=bg_sb[:, m:m + 1]
                    )
                    nc.scalar.activation(
                        u[:, m, :], pg_u[:, m, :], AF.Sigmoid, bias=bg_sb[:, 2 + m:3 + m]
                    )
                rh = wpool.tile([128, 2, NB], FP32, tag="rh")
                nc.vector.tensor_mul(rh[:], r[:], ht[:])
                return dict(b=b, xt=xt, ht=ht, sbc=sbc, u=u, rh=rh)

            def emit_h(st):
                b = st["b"]
                """h_hat matmul + tanh + final combine + store for block b."""
                ph = phpool.tile([128, 2, NB], FP32, tag="ph")
                for m in range(2):
                    for k in range(4):
                        act = st["xt"][:, k, :] if k < 2 else st["rh"][:, k - 2, :]
                        c = m * 4 + k
                        nc.tensor.matmul(
                            ph[:, m, :],
                            wh_sb[:, c * 128:(c + 1) * 128],
                            act,
                            start=(k == 0),
                            stop=(k == 3),
                        )
                hhat = wpool.tile([128, 2, NB], FP32, tag="hhat")
                for m in range(2):
                    nc.scalar.activation(
                        hhat[:, m, :], ph[:, m, :], AF.Tanh, bias=bh_sb[:, m:m + 1]
                    )
                e1 = wpool.tile([128, 2, NB], FP32, tag="e1")
                nc.vector.tensor_sub(e1[:], st["ht"][:], hhat[:])
                e2 = wpool.tile([128, 2, NB], FP32, tag="e2")
                nc.vector.tensor_mul(e2[:], st["u"][:], st["sbc"][:])
                t = wpool.tile([128, 2, NB], FP32, tag="t")
                nc.vector.tensor_mul(t[:], e1[:], e2[:])
                o = opool.tile([128, 2, NB], FP32, tag="o")
                nc.vector.tensor_add(o[:], t[:], hhat[:])
                nc.sync.dma_start(outTr[b], o[:])

            # Software-pipelined emission: block b's h-chain is emitted after
            # block b+1's gate matmuls so the PE never waits on the r*h
            # elementwise product. reps>1 repeats the whole pass (same
            # output) — used only for slope-based timing in bench.py.
            prev = None
            for _rep in range(reps):
                for b in range(NBLK):
                    st = emit_gates(b)
                    if prev is not None:
                        emit_h(prev)
                    prev = st
            emit_h(prev)

    nc.compile()
    return nc


def _get_nc():
    if "nc" not in _NC_CACHE:
        _NC_CACHE["nc"] = _build_nc()
    return _NC_CACHE["nc"]


def _pack_weights(W_r, W_u, W_h, b_r, b_u, b_h):
    wg = np.empty((128, 24 * 128), np.float32)
    for gi in range(4):
        W = W_r if gi < 2 else W_u
        m = gi % 2
        for k in range(6):
            c = gi * 6 + k
            wg[:, c * 128:(c + 1) * 128] = W[m * 128:(m + 1) * 128,
                                             k * 128:(k + 1) * 128].T
    wh = np.empty((128, 8 * 128), np.float32)
    for m in range(2):
        for k in range(4):
            c = m * 4 + k
            wh[:, c * 128:(c + 1) * 128] = W_h[m * 128:(m + 1) * 128,
                                               k * 128:(k + 1) * 128].T
    bg = np.stack([b_r[:128], b_r[128:], b_u[:128], b_u[128:]], axis=1)
    bh = np.stack([b_h[:128], b_h[128:]], axis=1)
    return (np.ascontiguousarray(wg), np.ascontiguousarray(wh),
            np.ascontiguousarray(bg), np.ascontiguousarray(bh))


def _make_in_maps(inputs, h_prev, attention_score, W_r, b_r, W_u, b_u, W_h, b_h):
    inputs = np.asarray(inputs, np.float32)
    h_prev = np.asarray(h_prev, np.float32)
    attention_score = np.asarray(attention_score, np.float32)
    wg, wh, bg, bh = _pack_weights(
        np.asarray(W_r, np.float32), np.asarray(W_u, np.float32),
        np.asarray(W_h, np.float32), np.asarray(b_r, np.float32),
        np.asarray(b_u, np.float32), np.asarray(b_h, np.float32),
    )
    in_maps = []
    for c in range(NCORES):
        sl = slice(c * BC, (c + 1) * BC)
        in_maps.append({
            "xT": np.ascontiguousarray(inputs[sl].T),
            "hT": np.ascontiguousarray(h_prev[sl].T),
            "sc": np.ascontiguousarray(attention_score[sl].reshape(NBLK, 1, NB)),
            "wg": wg, "wh": wh, "bg": bg, "bh": bh,
        })
    return in_maps


def _run(in_maps, trace=False, **kwargs):
    return run_bass_kernel_spmd(
        _get_nc(), in_maps, core_ids=list(range(NCORES)), trace=trace, **kwargs
    )


def _gather(results):
    out = np.empty((B, H), np.float32)
    for c in range(NCORES):
        out[c * BC:(c + 1) * BC] = results[c]["outT"].T
    return out


def kernel(**inputs):
    res = _run(_make_in_maps(**inputs), trace=False)
    return _gather(res.results)


# revision 17
# speedup vs baseline: 6.2440x; 3.2118x over previous
"""Trainium2 Bass kernel for a debiased GRU cell.

Computation (per batch row):
    r   = sigmoid(W_r @ [x; h] + b_r)
    u   = sigmoid(W_u @ [x; h] + b_u)
    hh  = tanh(W_h @ [x_int; r*h] + b_h)
    s   = score * u
    out = (1 - s) * hh + s * h

Strategy: data-parallel over 8 cores (8192 rows each). On-chip layout is
feature-major ([H, batch]) so that
  - activations never need an on-chip transpose (host supplies x.T / h.T),
  - gate biases fuse into the ACT engine's per-partition bias operand,
  - matmuls run with full K=128 / M=128 / N=512 tiles (PE at peak rate).
The only broadcast needed (attention score along partitions) runs on the
otherwise-idle GPSIMD engine. Output is produced as out.T and un-transposed
on the host.
"""

import numpy as np

import concourse.bacc as bacc
import concourse.bass as bass
import concourse.mybir as mybir
import concourse.tile as tile
from concourse.bass_utils import run_bass_kernel_spmd

B = 65536
I = 256
H = 256
NCORES = 8
BC = B // NCORES  # rows per core
NB = 512          # batch columns per block (max fp32 matmul free dim)
NBLK = BC // NB   # 16
FP32 = mybir.dt.float32
AF = mybir.ActivationFunctionType

_NC_CACHE = {}


def _build_nc(reps=1, loop=None, mm_dtype="fp32r"):
    nc = bacc.Bacc(
        "TRN2",
        target_bir_lowering=False,
        debug=False,
        enable_asserts=False,
    )

    # Matmul-operand dtype. float32r streams fp32 bits through the PE at
    # full rate (1 cycle/row vs 4 for plain fp32); walrus requires every
    # producer of an fp32r-matmul operand to declare an fp32r output, so
    # the whole feeding path (DRAM tensor -> DMA -> SBUF tile -> matmul)
    # is declared float32r. Bit layout is identical to fp32.
    MDT = {"fp32": mybir.dt.float32, "fp32r": mybir.dt.float32r}[mm_dtype]

    xT = nc.dram_tensor("xT", [2 * I, BC], MDT, kind="ExternalInput")
    hT = nc.dram_tensor("hT", [H, BC], MDT, kind="ExternalInput")
    sc = nc.dram_tensor("sc", [NBLK, 1, NB], FP32, kind="ExternalInput")
    wg = nc.dram_tensor("wg", [128, 24 * 128], MDT, kind="ExternalInput")
    wh = nc.dram_tensor("wh", [128, 8 * 128], MDT, kind="ExternalInput")
    bg = nc.dram_tensor("bg", [128, 4], FP32, kind="ExternalInput")
    bh = nc.dram_tensor("bh", [128, 2], FP32, kind="ExternalInput")
    outT = nc.dram_tensor("outT", [H, BC], FP32, kind="ExternalOutput")

    # [blk, partition, k-chunk, col]
    xTr = xT.rearrange("(k p) (b n) -> b p k n", p=128, n=NB)
    hTr = hT.rearrange("(k p) (b n) -> b p k n", p=128, n=NB)
    outTr = outT.rearrange("(m p) (b n) -> b p m n", p=128, n=NB)

    with tile.TileContext(nc) as tc:
        with (
            tc.tile_pool(name="const", bufs=1) as cpool,
            tc.tile_pool(name="xin", bufs=3) as xpool,
            tc.tile_pool(name="hin", bufs=3) as hpool,
            tc.tile_pool(name="sin", bufs=3) as spool,
            tc.tile_pool(name="gates", bufs=2) as gpool,
            tc.tile_pool(name="work", bufs=2) as wpool,
            tc.tile_pool(name="outp", bufs=2) as opool,
            tc.tile_pool(name="psg", bufs=3, space=bass.MemorySpace.PSUM) as pgpool,
            tc.tile_pool(name="psh", bufs=1, space=bass.MemorySpace.PSUM) as phpool,
        ):
            wg_sb = cpool.tile([128, 24 * 128], MDT)
            nc.sync.dma_start(wg_sb[:], wg[:])
            wh_sb = cpool.tile([128, 8 * 128], MDT)
            nc.sync.dma_start(wh_sb[:], wh[:])
            bg_sb = cpool.tile([128, 4], FP32)
            nc.sync.dma_start(bg_sb[:], bg[:])
            bh_sb = cpool.tile([128, 2], FP32)
            nc.sync.dma_start(bh_sb[:], bh[:])

            def emit_gates(b):
                """Load block b, run gate matmuls + sigmoids + r*h."""
                xt = xpool.tile([128, 4, NB], MDT, tag="xt")
                nc.sync.dma_start(xt[:], xTr[b])
                ht = hpool.tile([128, 2, NB], MDT, tag="ht")
                nc.sync.dma_start(ht[:], hTr[b])
                srow = spool.tile([1, NB], FP32, tag="srow")
                nc.sync.dma_start(srow[:], sc[b])
                sbc = spool.tile([128, 2, NB], FP32, tag="sbc")
                nc.gpsimd.partition_broadcast(sbc[:, 0, :], srow[:])
                nc.gpsimd.partition_broadcast(sbc[:, 1, :], srow[:])

                pg_r = pgpool.tile([128, 2, NB], FP32, tag="pg")
                pg_u = pgpool.tile([128, 2, NB], FP32, tag="pg")
                for gi in range(4):  # r0, r1, u0, u1
                    dst = (pg_r if gi < 2 else pg_u)[:, gi % 2, :]
                    for k in range(6):
                        act = xt[:, k, :] if k < 4 else ht[:, k - 4, :]
                        c = gi * 6 + k
                        nc.tensor.matmul(
                            dst,
                            wg_sb[:, c * 128:(c + 1) * 128],
                            act,
                            start=(k == 0),
                            stop=(k == 5),
                        )
                r = gpool.tile([128, 2, NB], FP32, tag="r")
                u = gpool.tile([128, 2, NB], FP32, tag="u")
                for m in range(2):
                    nc.scalar.activation(
                        r[:, m, :], pg_r[:, m, :], AF.Sigmoid, bias=bg_sb[:, m:m + 1]
                    )
                    nc.scalar.activation(
                        u[:, m, :], pg_u[:, m, :], AF.Sigmoid, bias=bg_sb[:, 2 + m:3 + m]
                    )
                rh = wpool.tile([128, 2, NB], MDT, tag="rh")
                nc.vector.tensor_mul(rh[:], r[:], ht[:])
                return dict(b=b, xt=xt, ht=ht, sbc=sbc, u=u, rh=rh)

            def emit_h(st):
                b = st["b"]
                """h_hat matmul + tanh + final combine + store for block b."""
                ph = phpool.tile([128, 2, NB], FP32, tag="ph")
                for m in range(2):
                    for k in range(4):
                        act = st["xt"][:, k, :] if k < 2 else st["rh"][:, k - 2, :]
                        c = m * 4 + k
                        nc.tensor.matmul(
                            ph[:, m, :],
                            wh_sb[:, c * 128:(c + 1) * 128],
                            act,
                            start=(k == 0),
                            stop=(k == 3),
                        )
                hhat = wpool.tile([128, 2, NB], FP32, tag="hhat")
                for m in range(2):
                    nc.scalar.activation(
                        hhat[:, m, :], ph[:, m, :], AF.Tanh, bias=bh_sb[:, m:m + 1]
                    )
                e1 = wpool.tile([128, 2, NB], FP32, tag="e1")
                nc.vector.tensor_sub(e1[:], st["ht"][:], hhat[:])
                e2 = wpool.tile([128, 2, NB], FP32, tag="e2")
                nc.vector.tensor_mul(e2[:], st["u"][:], st["sbc"][:])
                t = wpool.tile([128, 2, NB], FP32, tag="t")
                nc.vector.tensor_mul(t[:], e1[:], e2[:])
                o = opool.tile([128, 2, NB], FP32, tag="o")
                nc.vector.tensor_add(o[:], t[:], hhat[:])
                nc.sync.dma_start(outTr[b], o[:])

            # Software-pipelined emission: block b's h-chain is emitted after
            # block b+1's gate matmuls so the PE never waits on the r*h
            # elementwise product. reps>1 repeats the whole pass (same
            # output) — used only for slope-based timing in bench.py.
            def emit_pass():
                prev = None
                for _rep in range(reps):
                    for b in range(NBLK):
                        st = emit_gates(b)
                        if prev is not None:
                            emit_h(prev)
                        prev = st
                emit_h(prev)

            if loop is None:
                emit_pass()
            else:
                # bench-only: repeat the whole pass `loop` times inside one
                # NEFF execution for slope-based timing.
                with tc.For_i(0, loop, 1):
                    emit_pass()

    nc.compile()
    return nc


def _get_nc():
    if "nc" not in _NC_CACHE:
        _NC_CACHE["nc"] = _build_nc()
    return _NC_CACHE["nc"]


def _pack_weights(W_r, W_u, W_h, b_r, b_u, b_h):
    wg = np.empty((128, 24 * 128), np.float32)
    for gi in range(4):
        W = W_r if gi < 2 else W_u
        m = gi % 2
        for k in range(6):
            c = gi * 6 + k
            wg[:, c * 128:(c + 1) * 128] = W[m * 128:(m + 1) * 128,
                                             k * 128:(k + 1) * 128].T
    wh = np.empty((128, 8 * 128), np.float32)
    for m in range(2):
        for k in range(4):
            c = m * 4 + k
            wh[:, c * 128:(c + 1) * 128] = W_h[m * 128:(m + 1) * 128,
                                               k * 128:(k + 1) * 128].T
    bg = np.stack([b_r[:128], b_r[128:], b_u[:128], b_u[128:]], axis=1)
    bh = np.stack([b_h[:128], b_h[128:]], axis=1)
    return (np.ascontiguousarray(wg), np.ascontiguousarray(wh),
            np.ascontiguousarray(bg), np.ascontiguousarray(bh))


def _make_in_maps(inputs, h_prev, attention_score, W_r, b_r, W_u, b_u, W_h, b_h):
    inputs = np.asarray(inputs, np.float32)
    h_prev = np.asarray(h_prev, np.float32)
    attention_score = np.asarray(attention_score, np.float32)
    wg, wh, bg, bh = _pack_weights(
        np.asarray(W_r, np.float32), np.asarray(W_u, np.float32),
        np.asarray(W_h, np.float32), np.asarray(b_r, np.float32),
        np.asarray(b_u, np.float32), np.asarray(b_h, np.float32),
    )
    in_maps = []
    for c in range(NCORES):
        sl = slice(c * BC, (c + 1) * BC)
        in_maps.append({
            "xT": np.ascontiguousarray(inputs[sl].T),
            "hT": np.ascontiguousarray(h_prev[sl].T),
            "sc": np.ascontiguousarray(attention_score[sl].reshape(NBLK, 1, NB)),
            "wg": wg, "wh": wh, "bg": bg, "bh": bh,
        })
    return in_maps


def _run(in_maps, trace=False, **kwargs):
    return run_bass_kernel_spmd(
        _get_nc(), in_maps, core_ids=list(range(NCORES)), trace=trace, **kwargs
    )


def _gather(results):
    out = np.empty((B, H), np.float32)
    for c in range(NCORES):
        out[c * BC:(c + 1) * BC] = results[c]["outT"].T
    return out


def kernel(**inputs):
    res = _run(_make_in_maps(**inputs), trace=False)
    return _gather(res.results)
